# revision 33
# baseline (speedup 1.0000x reference)
"""ANI per-element MLP (MoE-routed) Trainium2 kernel, 8 NeuronCores.

Strategy
--------
The reference computes all 7 element MLPs for every atom and selects by
species (7x redundant).  Here the host routes instead: atoms are
stable-sorted by species, each species' atoms are split evenly across the
8 cores (identical padded group sizes G_s on every core, so one NEFF runs
SPMD), and each core runs its 7 dense per-species MLPs over contiguous
column groups.  No cross-core communication is needed (output is
per-atom).

Layout: activations are feature-major ([features(partition), atoms(free)])
so layers chain with zero transposes; the host supplies aev pre-transposed
as xT [1008, M] in bfloat16.  Matmuls run bf16 x bf16 -> fp32 PSUM at full
rate; all weight chunks are zero-padded to 128x128 on the host so every
matmul uses the whole PE array (no row/col-group sub-tiling, LDWEIGHTS
pull-ahead pipelines matmuls back-to-back).

CELU folding: substituting a = celu(z)/alpha + 1, 1/alpha is folded into
the weights (so PSUM holds z/alpha) and the per-layer shift into the bias
vector c = b/alpha - colsum(W); the device then computes per chunk
  e = Exp(ps + c)   [ScalarE]
  r = max(ps + c,0) [alternating ScalarE Relu / VectorE tensor_scalar]
  a = min(e,1) + r  [one fused VectorE scalar_tensor_tensor]
(exp overflow is safe: min(inf,1)=1 selects the relu branch exactly).
The final layer's alpha scale and bias are applied on the host during the
unpermute.

The emission order is a 4-stage software pipeline skewed across atom
tiles (L1(T), L2(T-2), L3(T-3), L4(T-4)) so the PE stream never waits on
the ScalarE/VectorE activation chain of the same tile.
"""

import os
import sys

import ml_dtypes
import numpy as np

BF16 = ml_dtypes.bfloat16

for _p in ("/opt/trn_rl_repo", "/root/.axon_site/_ro/trn_rl_repo"):
    if os.path.isdir(_p) and _p not in sys.path:
        sys.path.insert(0, _p)

import concourse.bass as bass
import concourse.bacc as bacc
import concourse.mybir as mybir
from concourse.bass_utils import run_bass_kernel_spmd
from concourse.tile import TileContext

N_CORES = 8
AEV_DIM = 1008
ALPHA = 0.1
INV_ALPHA = 10.0
NSPEC = 7
# hidden sizes per element net: H, C, N, O, S, F, Cl
LAYERS = [
    (256, 192, 160),
    (224, 192, 160),
    (192, 160, 128),
    (192, 160, 128),
    (160, 128, 96),
    (160, 128, 96),
    (160, 128, 96),
]
NTILE = 512  # atoms per matmul free-dim tile (fp32 moving-operand max)
KC = 126  # 1008 = 8 * 126 feature contraction chunks
KC1 = 127  # L1 chunks carry one extra row: ones (chunk 0) -> bias via matmul

# matmul dtype: "bf16" (half DMA traffic, ~0.7% rel err) or "f32r"
# (full fp32 storage, PE full-rate at N>=256, ~fp32-ish accuracy)
MM_DT = os.environ.get("ANI_MM_DT", "bf16")

LAST_RESULT = None
_CACHE: dict = {}


def _chunks(n):
    out = []
    while n > 0:
        c = min(128, n)
        out.append(c)
        n -= c
    return out


def _nch(n):
    return (n + 127) // 128


def _wblob_len():
    t = 0
    for h1, h2, h3 in LAYERS:
        t += 8 * KC1 * 128 * _nch(h1)
        t += 128 * _nch(h1) * 128 * _nch(h2)
        t += 128 * _nch(h2) * 128 * _nch(h3)
        t += 256
    return t


def _cblob_len():
    return NSPEC * 768


def _build(G):
    """Build the SPMD Bass program for per-core species group sizes G."""
    f32 = mybir.dt.float32
    bf16 = mybir.dt.bfloat16 if MM_DT == "bf16" else mybir.dt.float32r
    EXP = mybir.ActivationFunctionType.Exp
    RELU = mybir.ActivationFunctionType.Relu
    MIN = mybir.AluOpType.min
    MAX = mybir.AluOpType.max
    ADD = mybir.AluOpType.add

    M = int(sum(G))
    nc = bacc.Bacc()
    xt_d = nc.declare_dram_parameter("xt", [8 * KC1, M], bf16, isOutput=False)
    wb_d = nc.declare_dram_parameter("wb", [_wblob_len()], bf16, isOutput=False)
    cb_d = nc.declare_dram_parameter("cb", [_cblob_len()], f32, isOutput=False)
    out_d = nc.declare_dram_parameter("out", [M], f32, isOutput=True)
    xt_v = xt_d[:].rearrange("(k p) m -> p k m", p=KC1)

    with TileContext(nc) as tc:
        with (
            tc.tile_pool(name="wp", bufs=1) as wp,
            tc.tile_pool(name="xp", bufs=5) as xp,
            tc.tile_pool(name="tp", bufs=3) as tp,
            tc.tile_pool(name="ab", bufs=4) as ab,
            tc.tile_pool(name="yp", bufs=1) as yp,
            tc.tile_pool(name="pp", bufs=2, space="PSUM") as pp,
        ):
            y = yp.tile([1, M], f32, name="yrow", tag="yrow")

            def _sp_off(s):
                off = sum(
                    8 * KC1 * 128 * _nch(L[0])
                    + 128 * _nch(L[0]) * 128 * _nch(L[1])
                    + 128 * _nch(L[1]) * 128 * _nch(L[2])
                    + 256
                    for L in LAYERS[:s]
                )
                return off, s * 768

            def load_weights(s, off, coff, eng=None):
                """All inter-layer weights are zero-padded to full 128x128
                chunks on the host so every matmul uses the whole PE array
                (no row_grp/col_grp sub-tiling -> LDWEIGHTS pull-ahead
                works and matmuls pipeline back-to-back)."""
                h1, h2, h3 = LAYERS[s]
                if eng is None:
                    eng = nc.sync
                m1, k2, m2, k3, m3, k4 = _nch(h1), _nch(h1), _nch(h2), _nch(h2), _nch(h3), _nch(h3)
                w1 = wp.tile([KC1, 8, 128 * m1], bf16, name=f"w1s{s}", tag=f"w1s{s}")
                n1 = 8 * KC1 * 128 * m1
                eng.dma_start(
                    out=w1[:],
                    in_=wb_d[off : off + n1].rearrange(
                        "(k p h) -> p k h", p=KC1, h=128 * m1
                    ),
                )
                off += n1
                w2 = wp.tile([128, k2, 128 * m2], bf16, name=f"w2s{s}", tag=f"w2s{s}")
                n2 = 128 * k2 * 128 * m2
                eng.dma_start(
                    out=w2[:],
                    in_=wb_d[off : off + n2].rearrange(
                        "(k p h) -> p k h", p=128, h=128 * m2
                    ),
                )
                off += n2
                w3 = wp.tile([128, k3, 128 * m3], bf16, name=f"w3s{s}", tag=f"w3s{s}")
                n3 = 128 * k3 * 128 * m3
                eng.dma_start(
                    out=w3[:],
                    in_=wb_d[off : off + n3].rearrange(
                        "(k p h) -> p k h", p=128, h=128 * m3
                    ),
                )
                off += n3
                # W4 padded to [128, 2] bf16, single DMA
                w4 = wp.tile([128, 2], bf16, name=f"w4s{s}", tag=f"w4s{s}")
                eng.dma_start(
                    out=w4[:], in_=wb_d[off : off + 256].rearrange("(a p) -> p a", p=128)
                )
                off += 256
                # all biases for this species: [128, 6] f32, single DMA
                # (c1 chunks at cols 0:2, c2 at 2:4, c3 at 4:6; zero-padded)
                ct = wp.tile([128, 6], f32, name=f"cts{s}", tag=f"cts{s}")
                eng.dma_start(
                    out=ct[:], in_=cb_d[coff : coff + 768].rearrange("(a p) -> p a", p=128)
                )
                coff += 768
                return (w1, w2, w3, w4, ct[:, 0:2], ct[:, 2:4], ct[:, 4:6]), off, coff

            relu_rr = [0]  # alternate the Relu branch between ScalarE and VectorE

            def layer_merged(rhs, kp, nk, nm, w, nt, tagp):
                """L1 path: bias rides in the matmul (ones-row in rhs chunk 0,
                bias row in w), so the nm m-chunks share one wide Exp/Relu/STT
                op over a single [128, nm, NTILE] PSUM tile."""
                ps = pp.tile([128, 2, NTILE], f32, name=f"ps{tagp}", tag=f"ps{tagp}")
                for mi in range(nm):
                    for ki in range(nk):
                        nc.tensor.matmul(
                            ps[:, mi, 0:nt],
                            lhsT=w[0:kp, ki, mi * 128 : (mi + 1) * 128],
                            rhs=rhs[ki][0:kp, 0:nt],
                            start=(ki == 0),
                            stop=(ki == nk - 1),
                        )
                e = tp.tile([128, 2, NTILE], bf16, name="et", tag="et")
                r = tp.tile([128, 2, NTILE], bf16, name="rt", tag="rt")
                nc.scalar.activation(e[:, 0:nm, 0:nt], ps[:, 0:nm, 0:nt], EXP)
                relu_rr[0] += 1
                if relu_rr[0] % 2 == 0:
                    nc.scalar.activation(r[:, 0:nm, 0:nt], ps[:, 0:nm, 0:nt], RELU)
                else:
                    nc.vector.tensor_scalar(
                        r[:, 0:nm, 0:nt], ps[:, 0:nm, 0:nt], 0.0, None, MAX
                    )
                am = ab.tile([128, 2, NTILE], bf16, name=f"a{tagp}", tag=f"a{tagp}")
                nc.vector.scalar_tensor_tensor(
                    am[:, 0:nm, 0:nt], e[:, 0:nm, 0:nt], 1.0, r[:, 0:nm, 0:nt], MIN, ADD
                )
                return [am[:, mi, :] for mi in range(nm)]

            def layer(rhs, kp, nk, nm, w, ct, nt, tagp):
                """rhs: list of nk k-chunk APs ([kp or 128, nt]); returns nm
                activation tiles, all full 128 partitions (zero-padded
                weights make the pad rows exact zeros in PSUM).

                PSUM holds z/alpha (1/alpha folded into weights), so:
                  e = Exp(ps + c); r = max(ps + c, 0); a = min(e,1) + r
                """
                outs = []
                for mi in range(nm):
                    ps = pp.tile(
                        [128, NTILE], f32, name=f"ps{tagp}", tag=f"ps{tagp}",
                        bufs=1 if tagp == "3" else None,
                    )
                    for ki in range(nk):
                        nc.tensor.matmul(
                            ps[:, 0:nt],
                            lhsT=w[0:kp, ki, mi * 128 : (mi + 1) * 128],
                            rhs=rhs[ki][0:kp, 0:nt],
                            start=(ki == 0),
                            stop=(ki == nk - 1),
                        )
                    e = tp.tile([128, NTILE], bf16, name="et", tag="et")
                    r = tp.tile([128, NTILE], bf16, name="rt", tag="rt")
                    nc.scalar.activation(
                        e[:, 0:nt], ps[:, 0:nt], EXP,
                        bias=ct[:, mi : mi + 1], scale=1.0,
                    )
                    relu_rr[0] += 1
                    if relu_rr[0] % 2 == 0:
                        nc.scalar.activation(
                            r[:, 0:nt], ps[:, 0:nt], RELU,
                            bias=ct[:, mi : mi + 1], scale=1.0,
                        )
                    else:
                        nc.vector.tensor_scalar(
                            r[:, 0:nt], ps[:, 0:nt],
                            ct[:, mi : mi + 1], 0.0, ADD, MAX,
                        )
                    am = ab.tile([128, NTILE], bf16, name=f"a{tagp}{mi}", tag=f"a{tagp}{mi}")
                    nc.vector.scalar_tensor_tensor(
                        am[:, 0:nt], e[:, 0:nt], 1.0, r[:, 0:nt], MIN, ADD
                    )
                    outs.append(am)
                return outs

            # ---- flatten (species, tile) list; 4-stage skewed pipeline ----
            tiles = []  # (s, a0, nt, first_of_species)
            cols = [0] * NSPEC
            c = 0
            for s in range(NSPEC):
                cols[s] = c
                c += G[s]
            for s in range(NSPEC):
                sizes = []
                rem = G[s]
                while rem > 0:
                    c = min(NTILE, rem)
                    sizes.append(c)
                    rem -= c
                t0 = 0
                for ti, c in enumerate(sizes):
                    tiles.append((s, cols[s] + t0, c, ti == 0))
                    t0 += c

            WT = {}
            state = [dict() for _ in tiles]

            def st_l1(i):
                s, a0, nt, first = tiles[i]
                if first:
                    WT[s] = load_weights(s, *_sp_off(s))
                (w1, w2, w3, w4, c1, c2, c3), _, _ = WT[s]
                xt = xp.tile([KC1, 8, NTILE], bf16, name="xtile", tag="xtile")
                nc.sync.dma_start(out=xt[:, :, 0:nt], in_=xt_v[:, :, a0 : a0 + nt])
                state[i]["a1"] = layer_merged(
                    [xt[:, k, :] for k in range(8)], KC1, 8, _nch(LAYERS[s][0]), w1, nt, "1"
                )

            def st_l2(i):
                s, a0, nt, _ = tiles[i]
                (w1, w2, w3, w4, c1, c2, c3), _, _ = WT[s]
                h1, h2, h3 = LAYERS[s]
                state[i]["a2"] = layer(state[i].pop("a1"), 128, _nch(h1), _nch(h2), w2, c2, nt, "2")

            def st_l3(i):
                s, a0, nt, _ = tiles[i]
                (w1, w2, w3, w4, c1, c2, c3), _, _ = WT[s]
                h1, h2, h3 = LAYERS[s]
                state[i]["a3"] = layer(state[i].pop("a2"), 128, _nch(h2), _nch(h3), w3, c3, nt, "3")

            def st_l4(i):
                s, a0, nt, _ = tiles[i]
                (w1, w2, w3, w4, c1, c2, c3), _, _ = WT[s]
                nk4 = _nch(LAYERS[s][2])
                a3 = state[i].pop("a3")
                p4 = pp.tile([1, NTILE], f32, name="p4", tag="p4", bufs=1)
                for ki in range(nk4):
                    nc.tensor.matmul(
                        p4[0:1, 0:nt],
                        lhsT=w4[:, ki : ki + 1],
                        rhs=a3[ki][:, 0:nt],
                        start=(ki == 0),
                        stop=(ki == nk4 - 1),
                    )
                nc.any.tensor_copy(y[0:1, a0 : a0 + nt], p4[0:1, 0:nt])

            stages = ((st_l1, 0), (st_l2, 2), (st_l3, 3), (st_l4, 4))
            nt_total = len(tiles)
            for it in range(nt_total + 4):
                for stg, skew in stages:
                    ti = it - skew
                    if 0 <= ti < nt_total:
                        stg(ti)
            nc.sync.dma_start(out=out_d[:].rearrange("(o m) -> o m", o=1), in_=y[0:1, :])
    nc.finalize()
    return nc


def kernel(species, aev, params):
    global LAST_RESULT
    species = np.asarray(species).astype(np.int64)
    aev = np.asarray(aev, dtype=np.float32)
    n_atoms = species.shape[0]

    # ---- fold CELU alpha into weights/biases ----
    wparts, cparts = [], []
    b4p = np.zeros(NSPEC, np.float32)

    def padvec(v):
        out = np.zeros(256, np.float32)
        out[: v.shape[0]] = v
        return out

    def padw(W, kp, mp):
        """[k, m] -> zero-padded [kp, mp], flattened."""
        out = np.zeros((kp, mp), np.float32)
        out[: W.shape[0], : W.shape[1]] = W
        return out.ravel()

    for s in range(NSPEC):
        W1, b1, W2, b2, W3, b3, W4, b4 = [np.asarray(t, dtype=np.float32) for t in params[s]]
        h1, h2, h3 = LAYERS[s]
        W1p = np.zeros((8 * KC1, 128 * _nch(h1)), np.float32)
        for k in range(8):
            W1p[k * KC1 : k * KC1 + KC, :h1] = INV_ALPHA * W1[k * KC : (k + 1) * KC]
        W1p[KC, :h1] = INV_ALPHA * b1  # bias row (pairs with ones-row in chunk 0)
        wparts += [
            W1p.ravel(),
            padw(W2, 128 * _nch(h1), 128 * _nch(h2)),
            padw(W3, 128 * _nch(h2), 128 * _nch(h3)),
            padvec(W4[:, 0]),
        ]
        cparts += [
            padvec(INV_ALPHA * b1),
            padvec(INV_ALPHA * b2 - W2.sum(axis=0)),
            padvec(INV_ALPHA * b3 - W3.sum(axis=0)),
        ]
        b4p[s] = b4[0] - ALPHA * W4.sum()
    mmdt = BF16 if MM_DT == "bf16" else np.float32
    wblob = np.ascontiguousarray(np.concatenate(wparts).astype(mmdt))
    cblob = np.ascontiguousarray(np.concatenate(cparts), dtype=np.float32)
    assert wblob.shape[0] == _wblob_len() and cblob.shape[0] == _cblob_len()

    # ---- route: stable-sort by species, split each species evenly over cores ----
    counts = np.bincount(species, minlength=NSPEC).astype(np.int64)
    # even group sizes: fp32r matmuls require an even moving free dim
    G = tuple((g + (g & 1)) for g in (int(-(-int(c) // N_CORES)) for c in counts))
    M = int(sum(G))
    order = np.argsort(species, kind="stable")
    sofs = np.concatenate([[0], np.cumsum(counts)])
    core_parts = [[] for _ in range(N_CORES)]  # (species, src_indices) per group
    for s in range(NSPEC):
        if counts[s] == 0:
            continue
        block = order[sofs[s] : sofs[s + 1]]
        q, r = divmod(int(counts[s]), N_CORES)
        pos = 0
        for c in range(N_CORES):
            n = q + (1 if c < r else 0)
            core_parts[c].append((s, block[pos : pos + n]))
            pos += n

    in_maps = []
    for c in range(N_CORES):
        Xc = np.zeros((M, AEV_DIM), mmdt)
        col = 0
        for s, ch in core_parts[c]:
            Xc[col : col + len(ch)] = aev[ch]
            col += G[s]
        xTf = Xc.T  # [1008, M] view
        xT = np.zeros((8 * KC1, M), mmdt)
        for k in range(8):
            xT[k * KC1 : k * KC1 + KC] = xTf[k * KC : (k + 1) * KC]
        xT[KC, :] = 1.0  # ones row in chunk 0 -> bias via matmul
        xT = np.ascontiguousarray(xT)
        in_maps.append({"xt": xT, "wb": wblob, "cb": cblob})

    # ---- build (cached), run ----
    ck = (G, MM_DT)
    if ck not in _CACHE:
        _CACHE[ck] = _build(G)
    nc = _CACHE[ck]
    trace = os.environ.get("BASS_KERNEL_TRACE", "0") == "1"
    res = run_bass_kernel_spmd(nc, in_maps, core_ids=list(range(N_CORES)), trace=trace)
    LAST_RESULT = res

    # ---- unpermute + final-layer bias ----
    out = np.zeros(n_atoms, np.float32)
    for c in range(N_CORES):
        yc = np.asarray(res.results[c]["out"], dtype=np.float32)
        col = 0
        for s, ch in core_parts[c]:
            out[ch] = ALPHA * yc[col : col + len(ch)] + b4p[s]
            col += G[s]
    return out


# revision 34
# speedup vs baseline: 7.6586x; 7.6586x over previous
"""ANI per-element MLP (MoE-routed) Trainium2 kernel, 8 NeuronCores.

Strategy
--------
The reference computes all 7 element MLPs for every atom and selects by
species (7x redundant).  Here the host routes instead: atoms are
stable-sorted by species, each species' atoms are split evenly across the
8 cores (identical padded group sizes G_s on every core, so one NEFF runs
SPMD), and each core runs its 7 dense per-species MLPs over contiguous
column groups.  No cross-core communication is needed (output is
per-atom).

Layout: activations are feature-major ([features(partition), atoms(free)])
so layers chain with zero transposes; the host supplies aev pre-transposed
as xT [1008, M] in bfloat16.  Matmuls run bf16 x bf16 -> fp32 PSUM at full
rate; all weight chunks are zero-padded to 128x128 on the host so every
matmul uses the whole PE array (no row/col-group sub-tiling, LDWEIGHTS
pull-ahead pipelines matmuls back-to-back).

CELU folding: substituting a = celu(z)/alpha + 1, 1/alpha is folded into
the weights (so PSUM holds z/alpha) and the per-layer shift into the bias
vector c = b/alpha - colsum(W); the device then computes per chunk
  e = Exp(ps + c)   [ScalarE]
  r = max(ps + c,0) [alternating ScalarE Relu / VectorE tensor_scalar]
  a = min(e,1) + r  [one fused VectorE scalar_tensor_tensor]
(exp overflow is safe: min(inf,1)=1 selects the relu branch exactly).
The final layer's alpha scale and bias are applied on the host during the
unpermute.

The emission order is a 4-stage software pipeline skewed across atom
tiles (L1(T), L2(T-2), L3(T-3), L4(T-4)) so the PE stream never waits on
the ScalarE/VectorE activation chain of the same tile.
"""

import os
import sys

import ml_dtypes
import numpy as np

BF16 = ml_dtypes.bfloat16

for _p in ("/opt/trn_rl_repo", "/root/.axon_site/_ro/trn_rl_repo"):
    if os.path.isdir(_p) and _p not in sys.path:
        sys.path.insert(0, _p)

import concourse.bass as bass
import concourse.bacc as bacc
import concourse.mybir as mybir
from concourse.bass_utils import run_bass_kernel_spmd
from concourse.tile import TileContext

N_CORES = 8
AEV_DIM = 1008
ALPHA = 0.1
INV_ALPHA = 10.0
NSPEC = 7
# hidden sizes per element net: H, C, N, O, S, F, Cl
LAYERS = [
    (256, 192, 160),
    (224, 192, 160),
    (192, 160, 128),
    (192, 160, 128),
    (160, 128, 96),
    (160, 128, 96),
    (160, 128, 96),
]
NTILE = 512  # atoms per matmul free-dim tile (fp32 moving-operand max)
KC = 126  # 1008 = 8 * 126 feature contraction chunks
KC1 = 128  # L1 chunks: 126 features + ones row (chunk 0, bias via matmul) + pad

# matmul dtype: "bf16" (half DMA traffic, ~0.7% rel err) or "f32r"
# (full fp32 storage, PE full-rate at N>=256, ~fp32-ish accuracy)
MM_DT = os.environ.get("ANI_MM_DT", "bf16")

LAST_RESULT = None
_CACHE: dict = {}


def _chunks(n):
    out = []
    while n > 0:
        c = min(128, n)
        out.append(c)
        n -= c
    return out


def _nch(n):
    return (n + 127) // 128


def _wblob_len():
    t = 0
    for h1, h2, h3 in LAYERS:
        t += 8 * KC1 * 128 * _nch(h1)
        t += 128 * _nch(h1) * 128 * _nch(h2)
        t += 128 * _nch(h2) * 128 * _nch(h3)
        t += 256
    return t


def _cblob_len():
    return NSPEC * 768


def _build(G):
    """Build the SPMD Bass program for per-core species group sizes G."""
    f32 = mybir.dt.float32
    bf16 = mybir.dt.bfloat16 if MM_DT == "bf16" else mybir.dt.float32r
    EXP = mybir.ActivationFunctionType.Exp
    RELU = mybir.ActivationFunctionType.Relu
    MIN = mybir.AluOpType.min
    MAX = mybir.AluOpType.max
    ADD = mybir.AluOpType.add

    M = int(sum(G))
    nc = bacc.Bacc()
    xt_d = nc.declare_dram_parameter("xt", [8 * KC1, M], bf16, isOutput=False)
    wb_d = nc.declare_dram_parameter("wb", [_wblob_len()], bf16, isOutput=False)
    cb_d = nc.declare_dram_parameter("cb", [_cblob_len()], f32, isOutput=False)
    out_d = nc.declare_dram_parameter("out", [M], f32, isOutput=True)
    xt_v = xt_d[:].rearrange("(k p) m -> p k m", p=KC1)

    with TileContext(nc) as tc:
        with (
            tc.tile_pool(name="wp", bufs=1) as wp,
            tc.tile_pool(name="xp", bufs=5) as xp,
            tc.tile_pool(name="tp", bufs=3) as tp,
            tc.tile_pool(name="ab", bufs=4) as ab,
            tc.tile_pool(name="yp", bufs=1) as yp,
            tc.tile_pool(name="pp", bufs=2, space="PSUM") as pp,
        ):
            y = yp.tile([1, M], f32, name="yrow", tag="yrow")

            def _sp_off(s):
                off = sum(
                    8 * KC1 * 128 * _nch(L[0])
                    + 128 * _nch(L[0]) * 128 * _nch(L[1])
                    + 128 * _nch(L[1]) * 128 * _nch(L[2])
                    + 256
                    for L in LAYERS[:s]
                )
                return off, s * 768

            def load_weights(s, off, coff, eng=None):
                """All inter-layer weights are zero-padded to full 128x128
                chunks on the host so every matmul uses the whole PE array
                (no row_grp/col_grp sub-tiling -> LDWEIGHTS pull-ahead
                works and matmuls pipeline back-to-back)."""
                h1, h2, h3 = LAYERS[s]
                if eng is None:
                    eng = nc.sync
                m1, k2, m2, k3, m3, k4 = _nch(h1), _nch(h1), _nch(h2), _nch(h2), _nch(h3), _nch(h3)
                w1 = wp.tile([KC1, 8, 128 * m1], bf16, name=f"w1s{s}", tag=f"w1s{s}")
                n1 = 8 * KC1 * 128 * m1
                eng.dma_start(
                    out=w1[:],
                    in_=wb_d[off : off + n1].rearrange(
                        "(k p h) -> p k h", p=KC1, h=128 * m1
                    ),
                )
                off += n1
                w2 = wp.tile([128, k2, 128 * m2], bf16, name=f"w2s{s}", tag=f"w2s{s}")
                n2 = 128 * k2 * 128 * m2
                eng.dma_start(
                    out=w2[:],
                    in_=wb_d[off : off + n2].rearrange(
                        "(k p h) -> p k h", p=128, h=128 * m2
                    ),
                )
                off += n2
                w3 = wp.tile([128, k3, 128 * m3], bf16, name=f"w3s{s}", tag=f"w3s{s}")
                n3 = 128 * k3 * 128 * m3
                eng.dma_start(
                    out=w3[:],
                    in_=wb_d[off : off + n3].rearrange(
                        "(k p h) -> p k h", p=128, h=128 * m3
                    ),
                )
                off += n3
                # W4 padded to [128, 2] bf16, single DMA
                w4 = wp.tile([128, 2], bf16, name=f"w4s{s}", tag=f"w4s{s}")
                eng.dma_start(
                    out=w4[:], in_=wb_d[off : off + 256].rearrange("(a p) -> p a", p=128)
                )
                off += 256
                # all biases for this species: [128, 6] f32, single DMA
                # (c1 chunks at cols 0:2, c2 at 2:4, c3 at 4:6; zero-padded)
                ct = wp.tile([128, 6], f32, name=f"cts{s}", tag=f"cts{s}")
                eng.dma_start(
                    out=ct[:], in_=cb_d[coff : coff + 768].rearrange("(a p) -> p a", p=128)
                )
                coff += 768
                return (w1, w2, w3, w4, ct[:, 0:2], ct[:, 2:4], ct[:, 4:6]), off, coff

            relu_rr = [0]  # alternate the Relu branch between ScalarE and VectorE

            def layer_merged(rhs, kp, nk, nm, w, nt, tagp):
                """L1 path: bias rides in the matmul (ones-row in rhs chunk 0,
                bias row in w), so the nm m-chunks share one wide Exp/Relu/STT
                op over a single [128, nm, NTILE] PSUM tile."""
                ps = pp.tile([128, 2, NTILE], f32, name=f"ps{tagp}", tag=f"ps{tagp}")
                for mi in range(nm):
                    for ki in range(nk):
                        nc.tensor.matmul(
                            ps[:, mi, 0:nt],
                            lhsT=w[0:kp, ki, mi * 128 : (mi + 1) * 128],
                            rhs=rhs[ki][0:kp, 0:nt],
                            start=(ki == 0),
                            stop=(ki == nk - 1),
                        )
                e = tp.tile([128, 2, NTILE], bf16, name="et", tag="et")
                r = tp.tile([128, 2, NTILE], bf16, name="rt", tag="rt")
                nc.scalar.activation(e[:, 0:nm, 0:nt], ps[:, 0:nm, 0:nt], EXP)
                relu_rr[0] += 1
                if relu_rr[0] % 2 == 0:
                    nc.scalar.activation(r[:, 0:nm, 0:nt], ps[:, 0:nm, 0:nt], RELU)
                else:
                    nc.vector.tensor_scalar(
                        r[:, 0:nm, 0:nt], ps[:, 0:nm, 0:nt], 0.0, None, MAX
                    )
                am = ab.tile([128, 2, NTILE], bf16, name=f"a{tagp}", tag=f"a{tagp}")
                nc.vector.scalar_tensor_tensor(
                    am[:, 0:nm, 0:nt], e[:, 0:nm, 0:nt], 1.0, r[:, 0:nm, 0:nt], MIN, ADD
                )
                return [am[:, mi, :] for mi in range(nm)]

            def layer(rhs, kp, nk, nm, w, ct, nt, tagp):
                """rhs: list of nk k-chunk APs ([kp or 128, nt]); returns nm
                activation tiles, all full 128 partitions (zero-padded
                weights make the pad rows exact zeros in PSUM).

                PSUM holds z/alpha (1/alpha folded into weights), so:
                  e = Exp(ps + c); r = max(ps + c, 0); a = min(e,1) + r
                """
                outs = []
                for mi in range(nm):
                    ps = pp.tile(
                        [128, NTILE], f32, name=f"ps{tagp}", tag=f"ps{tagp}",
                        bufs=1 if tagp == "3" else None,
                    )
                    for ki in range(nk):
                        nc.tensor.matmul(
                            ps[:, 0:nt],
                            lhsT=w[0:kp, ki, mi * 128 : (mi + 1) * 128],
                            rhs=rhs[ki][0:kp, 0:nt],
                            start=(ki == 0),
                            stop=(ki == nk - 1),
                        )
                    e = tp.tile([128, NTILE], bf16, name="et", tag="et")
                    r = tp.tile([128, NTILE], bf16, name="rt", tag="rt")
                    nc.scalar.activation(
                        e[:, 0:nt], ps[:, 0:nt], EXP,
                        bias=ct[:, mi : mi + 1], scale=1.0,
                    )
                    relu_rr[0] += 1
                    if relu_rr[0] % 2 == 0:
                        nc.scalar.activation(
                            r[:, 0:nt], ps[:, 0:nt], RELU,
                            bias=ct[:, mi : mi + 1], scale=1.0,
                        )
                    else:
                        nc.vector.tensor_scalar(
                            r[:, 0:nt], ps[:, 0:nt],
                            ct[:, mi : mi + 1], 0.0, ADD, MAX,
                        )
                    am = ab.tile([128, NTILE], bf16, name=f"a{tagp}{mi}", tag=f"a{tagp}{mi}")
                    nc.vector.scalar_tensor_tensor(
                        am[:, 0:nt], e[:, 0:nt], 1.0, r[:, 0:nt], MIN, ADD
                    )
                    outs.append(am)
                return outs

            # ---- flatten (species, tile) list; 4-stage skewed pipeline ----
            tiles = []  # (s, a0, nt, first_of_species)
            cols = [0] * NSPEC
            c = 0
            for s in range(NSPEC):
                cols[s] = c
                c += G[s]
            for s in range(NSPEC):
                sizes = []
                rem = G[s]
                while rem > 0:
                    c = min(NTILE, rem)
                    sizes.append(c)
                    rem -= c
                t0 = 0
                for ti, c in enumerate(sizes):
                    tiles.append((s, cols[s] + t0, c, ti == 0))
                    t0 += c

            WT = {}
            state = [dict() for _ in tiles]

            def st_l1(i):
                s, a0, nt, first = tiles[i]
                if first:
                    WT[s] = load_weights(s, *_sp_off(s))
                (w1, w2, w3, w4, c1, c2, c3), _, _ = WT[s]
                xt = xp.tile([KC1, 8, NTILE], bf16, name="xtile", tag="xtile")
                nc.sync.dma_start(out=xt[:, :, 0:nt], in_=xt_v[:, :, a0 : a0 + nt])
                state[i]["a1"] = layer_merged(
                    [xt[:, k, :] for k in range(8)], KC1, 8, _nch(LAYERS[s][0]), w1, nt, "1"
                )

            def st_l2(i):
                s, a0, nt, _ = tiles[i]
                (w1, w2, w3, w4, c1, c2, c3), _, _ = WT[s]
                h1, h2, h3 = LAYERS[s]
                state[i]["a2"] = layer(state[i].pop("a1"), 128, _nch(h1), _nch(h2), w2, c2, nt, "2")

            def st_l3(i):
                s, a0, nt, _ = tiles[i]
                (w1, w2, w3, w4, c1, c2, c3), _, _ = WT[s]
                h1, h2, h3 = LAYERS[s]
                state[i]["a3"] = layer(state[i].pop("a2"), 128, _nch(h2), _nch(h3), w3, c3, nt, "3")

            def st_l4(i):
                s, a0, nt, _ = tiles[i]
                (w1, w2, w3, w4, c1, c2, c3), _, _ = WT[s]
                nk4 = _nch(LAYERS[s][2])
                a3 = state[i].pop("a3")
                p4 = pp.tile([1, NTILE], f32, name="p4", tag="p4", bufs=1)
                for ki in range(nk4):
                    nc.tensor.matmul(
                        p4[0:1, 0:nt],
                        lhsT=w4[:, ki : ki + 1],
                        rhs=a3[ki][:, 0:nt],
                        start=(ki == 0),
                        stop=(ki == nk4 - 1),
                    )
                nc.any.tensor_copy(y[0:1, a0 : a0 + nt], p4[0:1, 0:nt])

            stages = ((st_l1, 0), (st_l2, 2), (st_l3, 3), (st_l4, 4))
            nt_total = len(tiles)
            for it in range(nt_total + 4):
                for stg, skew in stages:
                    ti = it - skew
                    if 0 <= ti < nt_total:
                        stg(ti)
            nc.sync.dma_start(out=out_d[:].rearrange("(o m) -> o m", o=1), in_=y[0:1, :])
    nc.finalize()
    return nc


def kernel(species, aev, params):
    global LAST_RESULT
    species = np.asarray(species).astype(np.int64)
    aev = np.asarray(aev, dtype=np.float32)
    n_atoms = species.shape[0]

    # ---- fold CELU alpha into weights/biases ----
    wparts, cparts = [], []
    b4p = np.zeros(NSPEC, np.float32)

    def padvec(v):
        out = np.zeros(256, np.float32)
        out[: v.shape[0]] = v
        return out

    def padw(W, kp, mp):
        """[k, m] -> zero-padded [kp, mp], flattened."""
        out = np.zeros((kp, mp), np.float32)
        out[: W.shape[0], : W.shape[1]] = W
        return out.ravel()

    for s in range(NSPEC):
        W1, b1, W2, b2, W3, b3, W4, b4 = [np.asarray(t, dtype=np.float32) for t in params[s]]
        h1, h2, h3 = LAYERS[s]
        W1p = np.zeros((8 * KC1, 128 * _nch(h1)), np.float32)
        for k in range(8):
            W1p[k * KC1 : k * KC1 + KC, :h1] = INV_ALPHA * W1[k * KC : (k + 1) * KC]
        W1p[KC, :h1] = INV_ALPHA * b1  # bias row (pairs with ones-row in chunk 0)
        wparts += [
            W1p.ravel(),
            padw(W2, 128 * _nch(h1), 128 * _nch(h2)),
            padw(W3, 128 * _nch(h2), 128 * _nch(h3)),
            padvec(W4[:, 0]),
        ]
        cparts += [
            padvec(INV_ALPHA * b1),
            padvec(INV_ALPHA * b2 - W2.sum(axis=0)),
            padvec(INV_ALPHA * b3 - W3.sum(axis=0)),
        ]
        b4p[s] = b4[0] - ALPHA * W4.sum()
    mmdt = BF16 if MM_DT == "bf16" else np.float32
    wblob = np.ascontiguousarray(np.concatenate(wparts).astype(mmdt))
    cblob = np.ascontiguousarray(np.concatenate(cparts), dtype=np.float32)
    assert wblob.shape[0] == _wblob_len() and cblob.shape[0] == _cblob_len()

    # ---- route: stable-sort by species, split each species evenly over cores ----
    counts = np.bincount(species, minlength=NSPEC).astype(np.int64)
    # even group sizes: fp32r matmuls require an even moving free dim
    G = tuple((g + (g & 1)) for g in (int(-(-int(c) // N_CORES)) for c in counts))
    M = int(sum(G))
    order = np.argsort(species, kind="stable")
    sofs = np.concatenate([[0], np.cumsum(counts)])
    core_parts = [[] for _ in range(N_CORES)]  # (species, src_indices) per group
    for s in range(NSPEC):
        if counts[s] == 0:
            continue
        block = order[sofs[s] : sofs[s + 1]]
        q, r = divmod(int(counts[s]), N_CORES)
        pos = 0
        for c in range(N_CORES):
            n = q + (1 if c < r else 0)
            core_parts[c].append((s, block[pos : pos + n]))
            pos += n

    in_maps = []
    for c in range(N_CORES):
        Xc = np.zeros((M, AEV_DIM), mmdt)
        col = 0
        for s, ch in core_parts[c]:
            Xc[col : col + len(ch)] = aev[ch]
            col += G[s]
        xTf = Xc.T  # [1008, M] view
        xT = np.zeros((8 * KC1, M), mmdt)
        for k in range(8):
            xT[k * KC1 : k * KC1 + KC] = xTf[k * KC : (k + 1) * KC]
        xT[KC, :] = 1.0  # ones row in chunk 0 -> bias via matmul
        xT = np.ascontiguousarray(xT)
        in_maps.append({"xt": xT, "wb": wblob, "cb": cblob})

    # ---- build (cached), run ----
    ck = (G, MM_DT)
    if ck not in _CACHE:
        _CACHE[ck] = _build(G)
    nc = _CACHE[ck]
    trace = os.environ.get("BASS_KERNEL_TRACE", "0") == "1"
    res = run_bass_kernel_spmd(nc, in_maps, core_ids=list(range(N_CORES)), trace=trace)
    LAST_RESULT = res

    # ---- unpermute + final-layer bias ----
    out = np.zeros(n_atoms, np.float32)
    for c in range(N_CORES):
        yc = np.asarray(res.results[c]["out"], dtype=np.float32)
        col = 0
        for s, ch in core_parts[c]:
            out[ch] = ALPHA * yc[col : col + len(ch)] + b4p[s]
            col += G[s]
    return out


# revision 35
# speedup vs baseline: 7.8742x; 1.0281x over previous
"""ANI per-element MLP (MoE-routed) Trainium2 kernel, 8 NeuronCores.

Strategy
--------
The reference computes all 7 element MLPs for every atom and selects by
species (7x redundant).  Here the host routes instead: atoms are
stable-sorted by species, each species' atoms are split evenly across the
8 cores (identical padded group sizes G_s on every core, so one NEFF runs
SPMD), and each core runs its 7 dense per-species MLPs over contiguous
column groups.  No cross-core communication is needed (output is
per-atom).

Layout: activations are feature-major ([features(partition), atoms(free)])
so layers chain with zero transposes; the host supplies aev pre-transposed
as xT [1008, M] in bfloat16.  Matmuls run bf16 x bf16 -> fp32 PSUM at full
rate; all weight chunks are zero-padded to 128x128 on the host so every
matmul uses the whole PE array (no row/col-group sub-tiling, LDWEIGHTS
pull-ahead pipelines matmuls back-to-back).

CELU folding: substituting a = celu(z)/alpha + 1, 1/alpha is folded into
the weights (so PSUM holds z/alpha) and the per-layer shift into the bias
vector c = b/alpha - colsum(W); the device then computes per chunk
  e = Exp(ps + c)   [ScalarE]
  r = max(ps + c,0) [alternating ScalarE Relu / VectorE tensor_scalar]
  a = min(e,1) + r  [one fused VectorE scalar_tensor_tensor]
(exp overflow is safe: min(inf,1)=1 selects the relu branch exactly).
The final layer's alpha scale and bias are applied on the host during the
unpermute.

The emission order is a 4-stage software pipeline skewed across atom
tiles (L1(T), L2(T-2), L3(T-3), L4(T-4)) so the PE stream never waits on
the ScalarE/VectorE activation chain of the same tile.
"""

import os
import sys

import ml_dtypes
import numpy as np

BF16 = ml_dtypes.bfloat16

for _p in ("/opt/trn_rl_repo", "/root/.axon_site/_ro/trn_rl_repo"):
    if os.path.isdir(_p) and _p not in sys.path:
        sys.path.insert(0, _p)

import concourse.bass as bass
import concourse.bacc as bacc
import concourse.mybir as mybir
from concourse.bass_utils import run_bass_kernel_spmd
from concourse.tile import TileContext

N_CORES = 8
AEV_DIM = 1008
ALPHA = 0.1
INV_ALPHA = 10.0
NSPEC = 7
# hidden sizes per element net: H, C, N, O, S, F, Cl
LAYERS = [
    (256, 192, 160),
    (224, 192, 160),
    (192, 160, 128),
    (192, 160, 128),
    (160, 128, 96),
    (160, 128, 96),
    (160, 128, 96),
]
NTILE = 512  # atoms per matmul free-dim tile (fp32 moving-operand max)
KC = 126  # 1008 = 8 * 126 contraction chunks

# matmul dtype: "bf16" (half DMA traffic, ~0.7% rel err) or "f32r"
# (full fp32 storage, PE full-rate at N>=256, ~fp32-ish accuracy)
MM_DT = os.environ.get("ANI_MM_DT", "bf16")

LAST_RESULT = None
_CACHE: dict = {}


def _chunks(n):
    out = []
    while n > 0:
        c = min(128, n)
        out.append(c)
        n -= c
    return out


def _nch(n):
    return (n + 127) // 128


def _wblob_len():
    t = 0
    for h1, h2, h3 in LAYERS:
        t += AEV_DIM * 128 * _nch(h1)
        t += 128 * _nch(h1) * 128 * _nch(h2)
        t += 128 * _nch(h2) * 128 * _nch(h3)
        t += 256
    return t


def _cblob_len():
    return NSPEC * 768


def _build(G):
    """Build the SPMD Bass program for per-core species group sizes G."""
    f32 = mybir.dt.float32
    bf16 = mybir.dt.bfloat16 if MM_DT == "bf16" else mybir.dt.float32r
    EXP = mybir.ActivationFunctionType.Exp
    RELU = mybir.ActivationFunctionType.Relu
    MIN = mybir.AluOpType.min
    MAX = mybir.AluOpType.max
    ADD = mybir.AluOpType.add

    M = int(sum(G))
    nc = bacc.Bacc()
    xt_d = nc.declare_dram_parameter("xt", [AEV_DIM, M], bf16, isOutput=False)
    wb_d = nc.declare_dram_parameter("wb", [_wblob_len()], bf16, isOutput=False)
    cb_d = nc.declare_dram_parameter("cb", [_cblob_len()], f32, isOutput=False)
    out_d = nc.declare_dram_parameter("out", [M], f32, isOutput=True)
    xt_v = xt_d[:].rearrange("(k p) m -> p k m", p=KC)

    with TileContext(nc) as tc:
        with (
            tc.tile_pool(name="wp", bufs=1) as wp,
            tc.tile_pool(name="xp", bufs=5) as xp,
            tc.tile_pool(name="tp", bufs=3) as tp,
            tc.tile_pool(name="ab", bufs=4) as ab,
            tc.tile_pool(name="yp", bufs=1) as yp,
            tc.tile_pool(name="pp", bufs=2, space="PSUM") as pp,
        ):
            y = yp.tile([1, M], f32, name="yrow", tag="yrow")

            def _sp_off(s):
                off = sum(
                    AEV_DIM * 128 * _nch(L[0])
                    + 128 * _nch(L[0]) * 128 * _nch(L[1])
                    + 128 * _nch(L[1]) * 128 * _nch(L[2])
                    + 256
                    for L in LAYERS[:s]
                )
                return off, s * 768

            def load_weights(s, off, coff, eng=None):
                """All inter-layer weights are zero-padded to full 128x128
                chunks on the host so every matmul uses the whole PE array
                (no row_grp/col_grp sub-tiling -> LDWEIGHTS pull-ahead
                works and matmuls pipeline back-to-back)."""
                h1, h2, h3 = LAYERS[s]
                if eng is None:
                    eng = nc.sync
                m1, k2, m2, k3, m3, k4 = _nch(h1), _nch(h1), _nch(h2), _nch(h2), _nch(h3), _nch(h3)
                w1 = wp.tile([KC, 8, 128 * m1], bf16, name=f"w1s{s}", tag=f"w1s{s}")
                n1 = AEV_DIM * 128 * m1
                eng.dma_start(
                    out=w1[:],
                    in_=wb_d[off : off + n1].rearrange(
                        "(k p h) -> p k h", p=KC, h=128 * m1
                    ),
                )
                off += n1
                w2 = wp.tile([128, k2, 128 * m2], bf16, name=f"w2s{s}", tag=f"w2s{s}")
                n2 = 128 * k2 * 128 * m2
                eng.dma_start(
                    out=w2[:],
                    in_=wb_d[off : off + n2].rearrange(
                        "(k p h) -> p k h", p=128, h=128 * m2
                    ),
                )
                off += n2
                w3 = wp.tile([128, k3, 128 * m3], bf16, name=f"w3s{s}", tag=f"w3s{s}")
                n3 = 128 * k3 * 128 * m3
                eng.dma_start(
                    out=w3[:],
                    in_=wb_d[off : off + n3].rearrange(
                        "(k p h) -> p k h", p=128, h=128 * m3
                    ),
                )
                off += n3
                # W4 padded to [128, 2] bf16, single DMA
                w4 = wp.tile([128, 2], bf16, name=f"w4s{s}", tag=f"w4s{s}")
                eng.dma_start(
                    out=w4[:], in_=wb_d[off : off + 256].rearrange("(a p) -> p a", p=128)
                )
                off += 256
                # all biases for this species: [128, 6] f32, single DMA
                # (c1 chunks at cols 0:2, c2 at 2:4, c3 at 4:6; zero-padded)
                ct = wp.tile([128, 6], f32, name=f"cts{s}", tag=f"cts{s}")
                eng.dma_start(
                    out=ct[:], in_=cb_d[coff : coff + 768].rearrange("(a p) -> p a", p=128)
                )
                coff += 768
                return (w1, w2, w3, w4, ct[:, 0:2], ct[:, 2:4], ct[:, 4:6]), off, coff

            relu_rr = [0]  # alternate the Relu branch between ScalarE and VectorE

            def layer(rhs, kp, nk, nm, w, ct, nt, tagp):
                """rhs: list of nk k-chunk APs ([kp or 128, nt]); returns nm
                activation tiles, all full 128 partitions (zero-padded
                weights make the pad rows exact zeros in PSUM).

                PSUM holds z/alpha (1/alpha folded into weights), so:
                  e = Exp(ps + c); r = max(ps + c, 0); a = min(e,1) + r
                """
                outs = []
                for mi in range(nm):
                    ps = pp.tile([128, NTILE], f32, name=f"ps{tagp}", tag=f"ps{tagp}")
                    for ki in range(nk):
                        nc.tensor.matmul(
                            ps[:, 0:nt],
                            lhsT=w[0:kp, ki, mi * 128 : (mi + 1) * 128],
                            rhs=rhs[ki][0:kp, 0:nt],
                            start=(ki == 0),
                            stop=(ki == nk - 1),
                        )
                    e = tp.tile([128, NTILE], bf16, name="et", tag="et")
                    r = tp.tile([128, NTILE], bf16, name="rt", tag="rt")
                    nc.scalar.activation(
                        e[:, 0:nt], ps[:, 0:nt], EXP,
                        bias=ct[:, mi : mi + 1], scale=1.0,
                    )
                    relu_rr[0] += 1
                    if relu_rr[0] % 2 == 0:
                        nc.scalar.activation(
                            r[:, 0:nt], ps[:, 0:nt], RELU,
                            bias=ct[:, mi : mi + 1], scale=1.0,
                        )
                    else:
                        nc.vector.tensor_scalar(
                            r[:, 0:nt], ps[:, 0:nt],
                            ct[:, mi : mi + 1], 0.0, ADD, MAX,
                        )
                    am = ab.tile([128, NTILE], bf16, name=f"a{tagp}{mi}", tag=f"a{tagp}{mi}")
                    nc.vector.scalar_tensor_tensor(
                        am[:, 0:nt], e[:, 0:nt], 1.0, r[:, 0:nt], MIN, ADD
                    )
                    outs.append(am)
                return outs

            # ---- flatten (species, tile) list; 4-stage skewed pipeline ----
            tiles = []  # (s, a0, nt, first_of_species)
            cols = [0] * NSPEC
            c = 0
            for s in range(NSPEC):
                cols[s] = c
                c += G[s]
            for s in range(NSPEC):
                sizes = []
                rem = G[s]
                while rem > 0:
                    c = min(NTILE, rem)
                    sizes.append(c)
                    rem -= c
                t0 = 0
                for ti, c in enumerate(sizes):
                    tiles.append((s, cols[s] + t0, c, ti == 0))
                    t0 += c

            WT = {}
            state = [dict() for _ in tiles]

            def st_l1(i):
                s, a0, nt, first = tiles[i]
                if first:
                    WT[s] = load_weights(s, *_sp_off(s))
                (w1, w2, w3, w4, c1, c2, c3), _, _ = WT[s]
                xt = xp.tile([KC, 8, NTILE], bf16, name="xtile", tag="xtile")
                nc.sync.dma_start(out=xt[:, :, 0:nt], in_=xt_v[:, :, a0 : a0 + nt])
                state[i]["a1"] = layer(
                    [xt[:, k, :] for k in range(8)], KC, 8, _nch(LAYERS[s][0]), w1, c1, nt, "1"
                )

            def st_l2(i):
                s, a0, nt, _ = tiles[i]
                (w1, w2, w3, w4, c1, c2, c3), _, _ = WT[s]
                h1, h2, h3 = LAYERS[s]
                state[i]["a2"] = layer(state[i].pop("a1"), 128, _nch(h1), _nch(h2), w2, c2, nt, "2")

            def st_l3(i):
                s, a0, nt, _ = tiles[i]
                (w1, w2, w3, w4, c1, c2, c3), _, _ = WT[s]
                h1, h2, h3 = LAYERS[s]
                state[i]["a3"] = layer(state[i].pop("a2"), 128, _nch(h2), _nch(h3), w3, c3, nt, "3")

            def st_l4(i):
                s, a0, nt, _ = tiles[i]
                (w1, w2, w3, w4, c1, c2, c3), _, _ = WT[s]
                nk4 = _nch(LAYERS[s][2])
                a3 = state[i].pop("a3")
                p4 = pp.tile([1, NTILE], f32, name="p4", tag="p4")
                for ki in range(nk4):
                    nc.tensor.matmul(
                        p4[0:1, 0:nt],
                        lhsT=w4[:, ki : ki + 1],
                        rhs=a3[ki][:, 0:nt],
                        start=(ki == 0),
                        stop=(ki == nk4 - 1),
                    )
                nc.any.tensor_copy(y[0:1, a0 : a0 + nt], p4[0:1, 0:nt])

            stages = ((st_l1, 0), (st_l2, 2), (st_l3, 3), (st_l4, 4))
            nt_total = len(tiles)
            for it in range(nt_total + 4):
                for stg, skew in stages:
                    ti = it - skew
                    if 0 <= ti < nt_total:
                        stg(ti)
            nc.sync.dma_start(out=out_d[:].rearrange("(o m) -> o m", o=1), in_=y[0:1, :])
    nc.finalize()
    return nc


def kernel(species, aev, params):
    global LAST_RESULT
    species = np.asarray(species).astype(np.int64)
    aev = np.asarray(aev, dtype=np.float32)
    n_atoms = species.shape[0]

    # ---- fold CELU alpha into weights/biases ----
    wparts, cparts = [], []
    b4p = np.zeros(NSPEC, np.float32)

    def padvec(v):
        out = np.zeros(256, np.float32)
        out[: v.shape[0]] = v
        return out

    def padw(W, kp, mp):
        """[k, m] -> zero-padded [kp, mp], flattened."""
        out = np.zeros((kp, mp), np.float32)
        out[: W.shape[0], : W.shape[1]] = W
        return out.ravel()

    for s in range(NSPEC):
        W1, b1, W2, b2, W3, b3, W4, b4 = [np.asarray(t, dtype=np.float32) for t in params[s]]
        h1, h2, h3 = LAYERS[s]
        wparts += [
            padw(INV_ALPHA * W1, AEV_DIM, 128 * _nch(h1)),
            padw(W2, 128 * _nch(h1), 128 * _nch(h2)),
            padw(W3, 128 * _nch(h2), 128 * _nch(h3)),
            padvec(W4[:, 0]),
        ]
        cparts += [
            padvec(INV_ALPHA * b1),
            padvec(INV_ALPHA * b2 - W2.sum(axis=0)),
            padvec(INV_ALPHA * b3 - W3.sum(axis=0)),
        ]
        b4p[s] = b4[0] - ALPHA * W4.sum()
    mmdt = BF16 if MM_DT == "bf16" else np.float32
    wblob = np.ascontiguousarray(np.concatenate(wparts).astype(mmdt))
    cblob = np.ascontiguousarray(np.concatenate(cparts), dtype=np.float32)
    assert wblob.shape[0] == _wblob_len() and cblob.shape[0] == _cblob_len()

    # ---- route: stable-sort by species, split each species evenly over cores ----
    counts = np.bincount(species, minlength=NSPEC).astype(np.int64)
    # even group sizes: fp32r matmuls require an even moving free dim
    G = tuple((g + (g & 1)) for g in (int(-(-int(c) // N_CORES)) for c in counts))
    M = int(sum(G))
    order = np.argsort(species, kind="stable")
    sofs = np.concatenate([[0], np.cumsum(counts)])
    core_parts = [[] for _ in range(N_CORES)]  # (species, src_indices) per group
    for s in range(NSPEC):
        if counts[s] == 0:
            continue
        block = order[sofs[s] : sofs[s + 1]]
        q, r = divmod(int(counts[s]), N_CORES)
        pos = 0
        for c in range(N_CORES):
            n = q + (1 if c < r else 0)
            core_parts[c].append((s, block[pos : pos + n]))
            pos += n

    in_maps = []
    for c in range(N_CORES):
        Xc = np.zeros((M, AEV_DIM), mmdt)
        col = 0
        for s, ch in core_parts[c]:
            Xc[col : col + len(ch)] = aev[ch]
            col += G[s]
        xT = np.ascontiguousarray(Xc.T)
        in_maps.append({"xt": xT, "wb": wblob, "cb": cblob})

    # ---- build (cached), run ----
    ck = (G, MM_DT)
    if ck not in _CACHE:
        _CACHE[ck] = _build(G)
    nc = _CACHE[ck]
    trace = os.environ.get("BASS_KERNEL_TRACE", "0") == "1"
    res = run_bass_kernel_spmd(nc, in_maps, core_ids=list(range(N_CORES)), trace=trace)
    LAST_RESULT = res

    # ---- unpermute + final-layer bias ----
    out = np.zeros(n_atoms, np.float32)
    for c in range(N_CORES):
        yc = np.asarray(res.results[c]["out"], dtype=np.float32)
        col = 0
        for s, ch in core_parts[c]:
            out[ch] = ALPHA * yc[col : col + len(ch)] + b4p[s]
            col += G[s]
    return out


# revision 38
# speedup vs baseline: 8.0858x; 1.0269x over previous
"""ANI per-element MLP (MoE-routed) Trainium2 kernel, 8 NeuronCores.

Strategy
--------
The reference computes all 7 element MLPs for every atom and selects by
species (7x redundant).  Here the host routes instead: atoms are
stable-sorted by species, each species' atoms are split evenly across the
8 cores (identical padded group sizes G_s on every core, so one NEFF runs
SPMD), and each core runs its 7 dense per-species MLPs over contiguous
column groups.  No cross-core communication is needed (output is
per-atom).

Layout: activations are feature-major ([features(partition), atoms(free)])
so layers chain with zero transposes; the host supplies aev pre-transposed
as xT [1008, M] in bfloat16.  Matmuls run bf16 x bf16 -> fp32 PSUM at full
rate; all weight chunks are zero-padded to 128x128 on the host so every
matmul uses the whole PE array (no row/col-group sub-tiling, LDWEIGHTS
pull-ahead pipelines matmuls back-to-back).

CELU folding: substituting a = celu(z)/alpha + 1, 1/alpha is folded into
the weights (so PSUM holds z/alpha) and the per-layer shift into the bias
vector c = b/alpha - colsum(W); the device then computes per chunk
  e = Exp(ps + c)   [ScalarE]
  r = max(ps + c,0) [alternating ScalarE Relu / VectorE tensor_scalar]
  a = min(e,1) + r  [one fused VectorE scalar_tensor_tensor]
(exp overflow is safe: min(inf,1)=1 selects the relu branch exactly).
The final layer's alpha scale and bias are applied on the host during the
unpermute.

The emission order is a 4-stage software pipeline skewed across atom
tiles (L1(T), L2(T-2), L3(T-3), L4(T-4)) so the PE stream never waits on
the ScalarE/VectorE activation chain of the same tile.
"""

import os
import sys

import ml_dtypes
import numpy as np

BF16 = ml_dtypes.bfloat16

for _p in ("/opt/trn_rl_repo", "/root/.axon_site/_ro/trn_rl_repo"):
    if os.path.isdir(_p) and _p not in sys.path:
        sys.path.insert(0, _p)

import concourse.bass as bass
import concourse.bacc as bacc
import concourse.mybir as mybir
from concourse.bass_utils import run_bass_kernel_spmd
from concourse.tile import TileContext

N_CORES = 8
AEV_DIM = 1008
ALPHA = 0.1
INV_ALPHA = 10.0
NSPEC = 7
# hidden sizes per element net: H, C, N, O, S, F, Cl
LAYERS = [
    (256, 192, 160),
    (224, 192, 160),
    (192, 160, 128),
    (192, 160, 128),
    (160, 128, 96),
    (160, 128, 96),
    (160, 128, 96),
]
NTILE = 512  # atoms per matmul free-dim tile (fp32 moving-operand max)
KC = 126  # 1008 = 8 * 126 contraction chunks

# matmul dtype: "bf16" (half DMA traffic, ~0.7% rel err) or "f32r"
# (full fp32 storage, PE full-rate at N>=256, ~fp32-ish accuracy)
MM_DT = os.environ.get("ANI_MM_DT", "bf16")

LAST_RESULT = None
_CACHE: dict = {}


def _chunks(n):
    out = []
    while n > 0:
        c = min(128, n)
        out.append(c)
        n -= c
    return out


def _nch(n):
    return (n + 127) // 128


def _wblob_len():
    t = 0
    for h1, h2, h3 in LAYERS:
        t += AEV_DIM * 128 * _nch(h1)
        t += 128 * _nch(h1) * 128 * _nch(h2)
        t += 128 * _nch(h2) * 128 * _nch(h3)
        t += 256
    return t


def _cblob_len():
    return NSPEC * 768


def _build(G):
    """Build the SPMD Bass program for per-core species group sizes G."""
    f32 = mybir.dt.float32
    bf16 = mybir.dt.bfloat16 if MM_DT == "bf16" else mybir.dt.float32r
    EXP = mybir.ActivationFunctionType.Exp
    RELU = mybir.ActivationFunctionType.Relu
    MIN = mybir.AluOpType.min
    MAX = mybir.AluOpType.max
    ADD = mybir.AluOpType.add

    M = int(sum(G))
    nc = bacc.Bacc()
    xt_d = nc.declare_dram_parameter("xt", [AEV_DIM, M], bf16, isOutput=False)
    wb_d = nc.declare_dram_parameter("wb", [_wblob_len()], bf16, isOutput=False)
    cb_d = nc.declare_dram_parameter("cb", [_cblob_len()], f32, isOutput=False)
    out_d = nc.declare_dram_parameter("out", [M], f32, isOutput=True)
    xt_v = xt_d[:].rearrange("(k p) m -> p k m", p=KC)

    with TileContext(nc) as tc:
        with (
            tc.tile_pool(name="wp", bufs=1) as wp,
            tc.tile_pool(name="xp", bufs=5) as xp,
            tc.tile_pool(name="tp", bufs=4) as tp,
            tc.tile_pool(name="ab", bufs=5) as ab,
            tc.tile_pool(name="yp", bufs=1) as yp,
            tc.tile_pool(name="pp", bufs=2, space="PSUM") as pp,
        ):
            y = yp.tile([1, M], f32, name="yrow", tag="yrow")

            def _sp_off(s):
                off = sum(
                    AEV_DIM * 128 * _nch(L[0])
                    + 128 * _nch(L[0]) * 128 * _nch(L[1])
                    + 128 * _nch(L[1]) * 128 * _nch(L[2])
                    + 256
                    for L in LAYERS[:s]
                )
                return off, s * 768

            def load_weights(s, off, coff, eng=None):
                """All inter-layer weights are zero-padded to full 128x128
                chunks on the host so every matmul uses the whole PE array
                (no row_grp/col_grp sub-tiling -> LDWEIGHTS pull-ahead
                works and matmuls pipeline back-to-back)."""
                h1, h2, h3 = LAYERS[s]
                if eng is None:
                    eng = nc.sync
                m1, k2, m2, k3, m3, k4 = _nch(h1), _nch(h1), _nch(h2), _nch(h2), _nch(h3), _nch(h3)
                w1 = wp.tile([KC, 8, 128 * m1], bf16, name=f"w1s{s}", tag=f"w1s{s}")
                n1 = AEV_DIM * 128 * m1
                eng.dma_start(
                    out=w1[:],
                    in_=wb_d[off : off + n1].rearrange(
                        "(k p h) -> p k h", p=KC, h=128 * m1
                    ),
                )
                off += n1
                w2 = wp.tile([128, k2, 128 * m2], bf16, name=f"w2s{s}", tag=f"w2s{s}")
                n2 = 128 * k2 * 128 * m2
                eng.dma_start(
                    out=w2[:],
                    in_=wb_d[off : off + n2].rearrange(
                        "(k p h) -> p k h", p=128, h=128 * m2
                    ),
                )
                off += n2
                w3 = wp.tile([128, k3, 128 * m3], bf16, name=f"w3s{s}", tag=f"w3s{s}")
                n3 = 128 * k3 * 128 * m3
                eng.dma_start(
                    out=w3[:],
                    in_=wb_d[off : off + n3].rearrange(
                        "(k p h) -> p k h", p=128, h=128 * m3
                    ),
                )
                off += n3
                # W4 padded to [128, 2] bf16, single DMA
                w4 = wp.tile([128, 2], bf16, name=f"w4s{s}", tag=f"w4s{s}")
                eng.dma_start(
                    out=w4[:], in_=wb_d[off : off + 256].rearrange("(a p) -> p a", p=128)
                )
                off += 256
                # all biases for this species: [128, 6] f32, single DMA
                # (c1 chunks at cols 0:2, c2 at 2:4, c3 at 4:6; zero-padded)
                ct = wp.tile([128, 6], f32, name=f"cts{s}", tag=f"cts{s}")
                eng.dma_start(
                    out=ct[:], in_=cb_d[coff : coff + 768].rearrange("(a p) -> p a", p=128)
                )
                coff += 768
                return (w1, w2, w3, w4, ct[:, 0:2], ct[:, 2:4], ct[:, 4:6]), off, coff

            relu_rr = [0]  # alternate the Relu branch between ScalarE and VectorE

            def layer(rhs, kp, nk, nm, w, ct, nt, tagp):
                """rhs: list of nk k-chunk APs ([kp or 128, nt]); returns nm
                activation tiles, all full 128 partitions (zero-padded
                weights make the pad rows exact zeros in PSUM).

                PSUM holds z/alpha (1/alpha folded into weights), so:
                  e = Exp(ps + c); r = max(ps + c, 0); a = min(e,1) + r
                """
                outs = []
                for mi in range(nm):
                    ps = pp.tile([128, NTILE], f32, name=f"ps{tagp}", tag=f"ps{tagp}")
                    for ki in range(nk):
                        nc.tensor.matmul(
                            ps[:, 0:nt],
                            lhsT=w[0:kp, ki, mi * 128 : (mi + 1) * 128],
                            rhs=rhs[ki][0:kp, 0:nt],
                            start=(ki == 0),
                            stop=(ki == nk - 1),
                        )
                    e = tp.tile([128, NTILE], bf16, name="et", tag="et")
                    r = tp.tile([128, NTILE], bf16, name="rt", tag="rt")
                    nc.scalar.activation(
                        e[:, 0:nt], ps[:, 0:nt], EXP,
                        bias=ct[:, mi : mi + 1], scale=1.0,
                    )
                    relu_rr[0] += 1
                    if relu_rr[0] % 2 == 0:
                        nc.scalar.activation(
                            r[:, 0:nt], ps[:, 0:nt], RELU,
                            bias=ct[:, mi : mi + 1], scale=1.0,
                        )
                    else:
                        nc.vector.tensor_scalar(
                            r[:, 0:nt], ps[:, 0:nt],
                            ct[:, mi : mi + 1], 0.0, ADD, MAX,
                        )
                    am = ab.tile([128, NTILE], bf16, name=f"a{tagp}{mi}", tag=f"a{tagp}{mi}")
                    nc.vector.scalar_tensor_tensor(
                        am[:, 0:nt], e[:, 0:nt], 1.0, r[:, 0:nt], MIN, ADD
                    )
                    outs.append(am)
                return outs

            # ---- flatten (species, tile) list; 4-stage skewed pipeline ----
            tiles = []  # (s, a0, nt, first_of_species)
            cols = [0] * NSPEC
            c = 0
            for s in range(NSPEC):
                cols[s] = c
                c += G[s]
            for s in range(NSPEC):
                sizes = []
                rem = G[s]
                while rem > 0:
                    c = min(NTILE, rem)
                    sizes.append(c)
                    rem -= c
                t0 = 0
                for ti, c in enumerate(sizes):
                    tiles.append((s, cols[s] + t0, c, ti == 0))
                    t0 += c

            WT = {}
            state = [dict() for _ in tiles]

            def st_l1(i):
                s, a0, nt, first = tiles[i]
                if first:
                    WT[s] = load_weights(s, *_sp_off(s))
                (w1, w2, w3, w4, c1, c2, c3), _, _ = WT[s]
                xt = xp.tile([KC, 8, NTILE], bf16, name="xtile", tag="xtile")
                nc.sync.dma_start(out=xt[:, :, 0:nt], in_=xt_v[:, :, a0 : a0 + nt])
                state[i]["a1"] = layer(
                    [xt[:, k, :] for k in range(8)], KC, 8, _nch(LAYERS[s][0]), w1, c1, nt, "1"
                )

            def st_l2(i):
                s, a0, nt, _ = tiles[i]
                (w1, w2, w3, w4, c1, c2, c3), _, _ = WT[s]
                h1, h2, h3 = LAYERS[s]
                state[i]["a2"] = layer(state[i].pop("a1"), 128, _nch(h1), _nch(h2), w2, c2, nt, "2")

            def st_l3(i):
                s, a0, nt, _ = tiles[i]
                (w1, w2, w3, w4, c1, c2, c3), _, _ = WT[s]
                h1, h2, h3 = LAYERS[s]
                state[i]["a3"] = layer(state[i].pop("a2"), 128, _nch(h2), _nch(h3), w3, c3, nt, "3")

            def st_l4(i):
                s, a0, nt, _ = tiles[i]
                (w1, w2, w3, w4, c1, c2, c3), _, _ = WT[s]
                nk4 = _nch(LAYERS[s][2])
                a3 = state[i].pop("a3")
                p4 = pp.tile([1, NTILE], f32, name="p4", tag="p4")
                for ki in range(nk4):
                    nc.tensor.matmul(
                        p4[0:1, 0:nt],
                        lhsT=w4[:, ki : ki + 1],
                        rhs=a3[ki][:, 0:nt],
                        start=(ki == 0),
                        stop=(ki == nk4 - 1),
                    )
                nc.any.tensor_copy(y[0:1, a0 : a0 + nt], p4[0:1, 0:nt])

            stages = ((st_l1, 0), (st_l2, 2), (st_l3, 3), (st_l4, 4))
            nt_total = len(tiles)
            for it in range(nt_total + 4):
                for stg, skew in stages:
                    ti = it - skew
                    if 0 <= ti < nt_total:
                        stg(ti)
            nc.sync.dma_start(out=out_d[:].rearrange("(o m) -> o m", o=1), in_=y[0:1, :])
    nc.finalize()
    return nc


def kernel(species, aev, params):
    global LAST_RESULT
    species = np.asarray(species).astype(np.int64)
    aev = np.asarray(aev, dtype=np.float32)
    n_atoms = species.shape[0]

    # ---- fold CELU alpha into weights/biases ----
    wparts, cparts = [], []
    b4p = np.zeros(NSPEC, np.float32)

    def padvec(v):
        out = np.zeros(256, np.float32)
        out[: v.shape[0]] = v
        return out

    def padw(W, kp, mp):
        """[k, m] -> zero-padded [kp, mp], flattened."""
        out = np.zeros((kp, mp), np.float32)
        out[: W.shape[0], : W.shape[1]] = W
        return out.ravel()

    for s in range(NSPEC):
        W1, b1, W2, b2, W3, b3, W4, b4 = [np.asarray(t, dtype=np.float32) for t in params[s]]
        h1, h2, h3 = LAYERS[s]
        wparts += [
            padw(INV_ALPHA * W1, AEV_DIM, 128 * _nch(h1)),
            padw(W2, 128 * _nch(h1), 128 * _nch(h2)),
            padw(W3, 128 * _nch(h2), 128 * _nch(h3)),
            padvec(W4[:, 0]),
        ]
        cparts += [
            padvec(INV_ALPHA * b1),
            padvec(INV_ALPHA * b2 - W2.sum(axis=0)),
            padvec(INV_ALPHA * b3 - W3.sum(axis=0)),
        ]
        b4p[s] = b4[0] - ALPHA * W4.sum()
    mmdt = BF16 if MM_DT == "bf16" else np.float32
    wblob = np.ascontiguousarray(np.concatenate(wparts).astype(mmdt))
    cblob = np.ascontiguousarray(np.concatenate(cparts), dtype=np.float32)
    assert wblob.shape[0] == _wblob_len() and cblob.shape[0] == _cblob_len()

    # ---- route: stable-sort by species, split each species evenly over cores ----
    counts = np.bincount(species, minlength=NSPEC).astype(np.int64)
    # even group sizes: fp32r matmuls require an even moving free dim
    G = tuple((g + (g & 1)) for g in (int(-(-int(c) // N_CORES)) for c in counts))
    M = int(sum(G))
    order = np.argsort(species, kind="stable")
    sofs = np.concatenate([[0], np.cumsum(counts)])
    core_parts = [[] for _ in range(N_CORES)]  # (species, src_indices) per group
    for s in range(NSPEC):
        if counts[s] == 0:
            continue
        block = order[sofs[s] : sofs[s + 1]]
        q, r = divmod(int(counts[s]), N_CORES)
        pos = 0
        for c in range(N_CORES):
            n = q + (1 if c < r else 0)
            core_parts[c].append((s, block[pos : pos + n]))
            pos += n

    in_maps = []
    for c in range(N_CORES):
        Xc = np.zeros((M, AEV_DIM), mmdt)
        col = 0
        for s, ch in core_parts[c]:
            Xc[col : col + len(ch)] = aev[ch]
            col += G[s]
        xT = np.ascontiguousarray(Xc.T)
        in_maps.append({"xt": xT, "wb": wblob, "cb": cblob})

    # ---- build (cached), run ----
    ck = (G, MM_DT)
    if ck not in _CACHE:
        _CACHE[ck] = _build(G)
    nc = _CACHE[ck]
    trace = os.environ.get("BASS_KERNEL_TRACE", "0") == "1"
    res = run_bass_kernel_spmd(nc, in_maps, core_ids=list(range(N_CORES)), trace=trace)
    LAST_RESULT = res

    # ---- unpermute + final-layer bias ----
    out = np.zeros(n_atoms, np.float32)
    for c in range(N_CORES):
        yc = np.asarray(res.results[c]["out"], dtype=np.float32)
        col = 0
        for s, ch in core_parts[c]:
            out[ch] = ALPHA * yc[col : col + len(ch)] + b4p[s]
            col += G[s]
    return out
